# revision 25
# baseline (speedup 1.0000x reference)
# Trainium2 Bass kernel for CoAttentionModule (axial co-attention, 8 heads).
#
# Sharding: data-parallel over (direction, batch) = 2 x 4 = 8 NeuronCores.
# Core c computes weighted = _coattention(qf, rf)[b].T for its (d, b) pair;
# the host concatenates [features, weighted] per direction.
#
# On-chip layout: the hw axis is w-major everywhere (col = w*32 + i, i = h
# index); the host pre-permutes features and un-permutes the output. This
# makes every matmul stationary operand a contiguous SBUF slice (walrus
# requires single-free-dim weight APs).
#
# Per-core pipeline (bf16 matmul operands, fp32 PSUM accumulation; the K and
# V projections run in fp8e4m3 DoubleRow mode — 2 contraction k-tiles per
# pass, 2x MAC rate — with weights pre-scaled by 64 to clear the e4m3
# subnormal region; the 64x scale on k rides into the scores and is folded
# into the softmax temperature, and v is unscaled in its PSUM->SBUF copy):
#   qT = Wq.T @ xq (+bq)          [c_out, hw] all heads up front  (bf16)
#   QAUG[t', col(w,i)] = sum_c 64*rel_emb[(t'-i)%63, c] q[c, col]  (63 aug
#       rows, via 32 host-rolled copies of rel.T) — batched over all 8 heads
#       (moving free dim = 8 heads x 32 w), scattered to sqa_all on the ACT
#       engine so the PE never waits on the PSUM WAR.
#   per head:
#     kT = 64*Wk.T @ xr8 + 64*RWF  RWF[c,(w,k)] = rel_emb[(k-w)%63, c] (rel_w
#                                  folded into keys; bk cancels in softmax)
#     v  = (xr8.T @ 64*Wv)/64      [(w,k), c]                (fp8 DR; ACT copy)
#     scores tile (w-group of 4) [128=(w,i), 128=(w,k)]:
#        q.k' + QAUG.KAUG(one-hot) + WIND.KMASK(-1e30 off-diag mask channels)
#     softmax: exp(scale=1/(16*64)) with accum_out row sums -> recip -> scale
#     probsT via DVE 32x32 stream transpose (block-diag => exact transpose)
#     avT[c,(w,i)] = v.T @ probsT, software-pipelined so wg's softmax chain
#        hides behind wg+1's score matmuls
#   output proj outT = Wo.T @ attT + bo'
#   (bv folded on host: bo' = bv @ Wo + bo; bk dropped: softmax-invariant)
import numpy as np
import ml_dtypes

B, C, H, W = 4, 2048, 32, 32
HW = H * W
NH, HD = 8, 256
T = 2 * max(H, W) - 1  # 63
NC = C // 128  # 16 chunks
FS = np.float32(64.0)  # fp8 weight pre-scale
OSPLIT = 12  # output-proj contraction chunks (of 16) done in fp8 DoubleRow

_CACHE = {}


def _hostprep(Wq, bq, Wk, bk, Wv, bv, Wo, bo, rel_emb):
    bf = ml_dtypes.bfloat16
    f8 = ml_dtypes.float8_e4m3
    f32 = np.float32
    Wq, Wk, Wv, Wo = (np.asarray(a, f32) for a in (Wq, Wk, Wv, Wo))
    rel = np.asarray(rel_emb, f32)  # [63, 256]
    ii = np.arange(32)

    # lhsT blobs [co, p, ci*128+m]: one contiguous [128, 2048] DMA per co chunk
    def lchunks(Wm, dt, s=1.0):
        return np.ascontiguousarray(
            (Wm * s).reshape(NC, 128, NC, 128).transpose(2, 1, 0, 3).reshape(NC, 128, C)
        ).astype(dt)

    wq_l = lchunks(Wq, bf)
    wk_l = lchunks(Wk, f8, FS)
    wo_all = lchunks(Wo, np.float32, FS)
    wo8_l = np.ascontiguousarray(wo_all[:, :, :OSPLIT * 128]).astype(f8)
    wob_l = np.ascontiguousarray(wo_all[:, :, OSPLIT * 128:]).astype(bf)
    # V weights per head [n, p, ci*256+m]: one contiguous [128, 4096] DMA per head
    wv_r = np.ascontiguousarray(
        (Wv * FS).reshape(NC, 128, NH, HD).transpose(2, 1, 0, 3).reshape(NH, 128, NC * HD)
    ).astype(f8)

    bq_c = np.ascontiguousarray(np.asarray(bq, f32).reshape(NC, 128).T)  # [128,16]
    bo2 = np.asarray(bv, f32) @ Wo + np.asarray(bo, f32)
    bo2_c = np.ascontiguousarray(bo2.reshape(NC, 128).T)  # [128,16]

    w_idx, k_idx = np.meshgrid(np.arange(32), np.arange(32), indexing="ij")
    # rel_w fold table, w-major [2, 128, 1024]: rwf[ch, p, w*32+k] = 64*rel[(k-w)%63, ch*128+p]
    rwf = rel[(k_idx - w_idx) % T].reshape(HW, HD)  # [(w,k), 256]
    rwf = np.ascontiguousarray((rwf * FS).T.reshape(2, 128, HW)).astype(f32)
    # rolled rel_emb.T for QAUG: relroll[p, (i, ch, t')] = 64*rel[(t'-i)%63, ch*128+p]
    # t' padded 63->128 with zeros so the stationary operand is 128 wide (FWL)
    relroll = np.zeros((128, 32 * 2 * 128), f32)
    for i in range(32):
        for ch in range(2):
            blk = rel[(np.arange(T) - i) % T, ch * 128:(ch + 1) * 128]  # [63,128]
            relroll[:, (i * 2 + ch) * 128:(i * 2 + ch) * 128 + T] = blk.T * FS
    relroll = relroll.astype(bf)
    # key-side aug channels [96, 1024] w-major: rows 0:63 one-hot rel gather
    # (kaug[t, w*32+k] = t==k), row 63 zero, rows 64:96 block-diag mask
    # (kmask[w', w*32+k] = 0 if w==w' else -1e30). Query side: rows 0:63 QAUG,
    # row 63 zero, rows 64:96 w-indicator.
    kaug = np.zeros((96, HW), f32)
    kaug[k_idx.reshape(-1), np.arange(HW)] = 1.0
    kaug[64:96] = -1e30
    wind = np.zeros((32, HW), f32)
    for w in range(32):
        wind[w, w * 32 + ii] = 1.0  # query col w*32+i
        kaug[64 + w, w * 32 + ii] = 0.0  # key col w*32+k
    kaug = kaug.astype(bf)
    wind = wind.astype(bf)

    return dict(wq_l=wq_l, wk_l=wk_l, wo8_l=wo8_l, wob_l=wob_l, wv_r=wv_r,
                bq_c=bq_c, bo2_c=bo2_c, rwf=rwf, relroll=relroll, kaug=kaug,
                wind=wind)


def _build(timing_twin=False, loop=1, parts=("q", "k", "v", "qaug", "att", "o"), bare=False):
    import concourse.bacc as bacc
    import concourse.mybir as mybir
    import concourse.tile as tile

    parts = frozenset(parts) if not bare else frozenset()

    F32, BF16, F8 = mybir.dt.float32, mybir.dt.bfloat16, mybir.dt.float8e4
    DR = mybir.MatmulPerfMode.DoubleRow
    nc = bacc.Bacc(None, target_bir_lowering=False)

    if timing_twin:
        # timing-equivalent NEFF: big tensors live in internal DRAM scratch
        # (no per-call host staging), only a tiny external in/out pair.
        def declare(name, shape, dt, isOutput=False):
            return nc.dram_tensor(name, shape, dt)
        tiny_in = nc.declare_dram_parameter("tiny_in", [1, 4], F32, isOutput=False)
        tiny_out = nc.declare_dram_parameter("tiny_out", [1, 4], F32, isOutput=True)
    else:
        declare = nc.declare_dram_parameter

    xq = declare("xq", [C, HW], BF16, isOutput=False)
    xr8 = declare("xr8", [C, HW], F8, isOutput=False)
    wq_l = declare("wq_l", [NC, 128, C], BF16, isOutput=False)
    wk_l = declare("wk_l", [NC, 128, C], F8, isOutput=False)
    wo8_l = declare("wo8_l", [NC, 128, OSPLIT * 128], F8, isOutput=False)
    wob_l = declare("wob_l", [NC, 128, (NC - OSPLIT) * 128], BF16, isOutput=False)
    wv_r = declare("wv_r", [NH, 128, NC * HD], F8, isOutput=False)
    bq_c = declare("bq_c", [128, NC], F32, isOutput=False)
    bo2_c = declare("bo2_c", [128, NC], F32, isOutput=False)
    rwf = declare("rwf", [2, 128, HW], F32, isOutput=False)
    relroll = declare("relroll", [128, 32 * 2 * 128], BF16, isOutput=False)
    kaug = declare("kaug", [96, HW], BF16, isOutput=False)
    wind = declare("wind", [32, HW], BF16, isOutput=False)
    out = declare("out", [C, HW], BF16, isOutput=True)

    EXP = mybir.ActivationFunctionType.Exp

    with tile.TileContext(nc) as tc:
        with (
            tc.tile_pool(name="feat", bufs=2) as feat_pool,
            tc.tile_pool(name="att", bufs=1) as att_pool,
            tc.tile_pool(name="const", bufs=1) as const_pool,
            tc.tile_pool(name="head", bufs=2) as head_pool,
            tc.tile_pool(name="wstr", bufs=2) as wstr_pool,
            tc.tile_pool(name="probs", bufs=3) as probs_pool,
            tc.tile_pool(name="outs", bufs=3) as outs_pool,
            tc.tile_pool(name="psum", bufs=4, space="PSUM") as psum_pool,
            tc.tile_pool(name="psumb", bufs=2, space="PSUM") as psumb_pool,
        ):
            # ---- load features + constants (resident) ----
            # DMA issue order is the cold-start critical path: the first Q
            # group needs all xq chunks + wq[0] + c_bq, so those go first;
            # xr8/rwf (K, ~115us in) and roll/kaug/wind (QAUG/attention)
            # follow; c_bo (output proj) last.
            if bare:
                loop = 0
            if not bare:
                xqt = feat_pool.tile([128, NC * HW], BF16, tag="featq", bufs=1)
                xr8t = feat_pool.tile([128, NC * HW], F8, tag="featr", bufs=1)
                # interleave the first two Wq chunks into the xq stream so the
                # first Q group starts DMA-paced (~5us) instead of waiting for
                # weights queued behind all of xq (~14us)
                prestaged = []
                for cc in range(NC):
                    nc.sync.dma_start(xqt[:, cc * HW:(cc + 1) * HW], xq[cc * 128:(cc + 1) * 128, :])
                    if cc in (3, 8) and "q" in parts:
                        wt = wstr_pool.tile([128, C], BF16, tag="wl")
                        nc.sync.dma_start(wt[:], wq_l[len(prestaged)])
                        prestaged.append(wt)
                c_bq = const_pool.tile([128, NC], F32)
                nc.sync.dma_start(c_bq[:], bq_c[:])
                for cc in range(NC):
                    nc.sync.dma_start(xr8t[:, cc * HW:(cc + 1) * HW], xr8[cc * 128:(cc + 1) * 128, :])
                attT8 = att_pool.tile([128, OSPLIT * HW], F8, tag="attT8")
                attTb = att_pool.tile([128, (NC - OSPLIT) * HW], BF16, tag="attTb")
                if "att" not in parts:
                    nc.vector.memset(attT8[:], 0.0)
                    nc.vector.memset(attTb[:], 0.0)

                c_rwf = const_pool.tile([128, 2 * HW], F32)
                nc.sync.dma_start(c_rwf[:, 0:HW], rwf[0])
                nc.sync.dma_start(c_rwf[:, HW:2 * HW], rwf[1])
                c_roll = const_pool.tile([128, 32 * 2 * 128], BF16)
                nc.sync.dma_start(c_roll[:], relroll[:])
                c_kaug = const_pool.tile([96, HW], BF16)
                nc.sync.dma_start(c_kaug[:], kaug[:])
                c_wind = const_pool.tile([32, HW], BF16)
                nc.sync.dma_start(c_wind[:], wind[:])
                c_bo = const_pool.tile([128, NC], F32)
                nc.sync.dma_start(c_bo[:], bo2_c[:])

                # all-head q [c_out chunk-major, hw] and query-side aug rows
                # [96, head-major hw]; aug rows 63..96 are static (zero + w-ind)
                qall = att_pool.tile([128, NC * HW], BF16, tag="qall")
                sqa_all = att_pool.tile([96, NH * HW], BF16, tag="sqaall")
                nc.vector.memset(sqa_all[:], 0.0)
                for n in range(NH):
                    nc.vector.tensor_copy(sqa_all[64:96, n * HW:(n + 1) * HW], c_wind[:])

            # DoubleRow moving-operand slice: k-tile pair cp, columns
            # h2*512..(h2+1)*512 of each tile -> AP [128, 2, 512]
            def xr8_mov(cp, h2):
                return xr8t[:, cp * 2 * HW:(cp + 1) * 2 * HW].rearrange(
                    "p (t n) -> p t n", t=2)[:, :, h2 * 512:(h2 + 1) * 512]

            for rep in range(loop):
                # ---- Q projection phase (bf16): qall[co] = Wq.T @ xq + bq ----
                for co in range(NC if "q" in parts else 0):
                    if rep == 0 and co < len(prestaged):
                        wt = prestaged[co]
                    else:
                        wt = wstr_pool.tile([128, C], BF16, tag="wl")
                        nc.sync.dma_start(wt[:], wq_l[co])
                    for h2 in range(2):
                        ps = psum_pool.tile([128, 512], F32, tag="pp")
                        for ci in range(NC):
                            nc.tensor.matmul(
                                ps[:], wt[:, ci * 128:(ci + 1) * 128],
                                xqt[:, ci * HW + h2 * 512: ci * HW + (h2 + 1) * 512],
                                start=(ci == 0), stop=(ci == NC - 1))
                        nc.vector.tensor_scalar_add(
                            qall[:, co * HW + h2 * 512: co * HW + (h2 + 1) * 512],
                            ps[:], c_bq[:, co:co + 1])

                # ---- QAUG batched over heads: per query-row i, rolled
                # rel_emb.T contraction, 256-wide moving (8 heads x 32 w) ----
                if "qaug" in parts:
                    for ip in range(16):
                        pqa = psum_pool.tile([128, 512], F32, tag="pp")
                        for i2 in range(2):
                            i = ip * 2 + i2
                            for ch in range(2):
                                nc.tensor.matmul(
                                    pqa[:, i2 * 256:(i2 + 1) * 256],
                                    c_roll[:, (i * 2 + ch) * 128:(i * 2 + ch + 1) * 128],
                                    qall[:].rearrange("p (n2 ch w i) -> p n2 ch w i",
                                                      n2=NH, ch=2, w=32, i=32)[:, :, ch, :, i],
                                    start=(ch == 0), stop=(ch == 1))
                        # scatter [t, (i2, n, w)] -> sqa_all[t, n*HW + w*32 + i]
                        nc.scalar.activation(
                            sqa_all[0:T, :].rearrange(
                                "p (n w i) -> p n w i", n=NH, w=32, i=32)[
                                :, :, :, ip * 2:(ip + 1) * 2],
                            pqa[0:T, :].rearrange(
                                "p (i2 n w) -> p n w i2", i2=2, n=NH),
                            mybir.ActivationFunctionType.Copy)

                for n in range(NH):
                    sk = head_pool.tile([128, 2 * HW], BF16, tag="sk")
                    sv = head_pool.tile([128, NH * HD], BF16, tag="sv")
                    swv = head_pool.tile([128, NC * HD], F8, tag="swv")

                    # stage this head's V weights once (one 0.5MB DMA)
                    nc.sync.dma_start(swv[:], wv_r[n])

                    # ---- K projection (fp8 DoubleRow): 64*Wk.T @ xr8 + 64*RWF ----
                    for co2 in range(2 if "k" in parts else 0):
                        co = n * 2 + co2
                        wt8 = wstr_pool.tile([128, C], F8, tag="wl8")
                        nc.sync.dma_start(wt8[:], wk_l[co])
                        for h2 in range(2):
                            ps = psum_pool.tile([128, 512], F32, tag="pp")
                            for cp in range(8):
                                nc.tensor.matmul(
                                    ps[:],
                                    wt8[:, cp * 256:(cp + 1) * 256].rearrange(
                                        "p (t m) -> p t m", t=2),
                                    xr8_mov(cp, h2),
                                    start=(cp == 0), stop=(cp == 7), perf_mode=DR)
                            nc.vector.tensor_add(
                                sk[:, co2 * HW + h2 * 512: co2 * HW + (h2 + 1) * 512],
                                ps[:],
                                c_rwf[:, co2 * HW + h2 * 512: co2 * HW + (h2 + 1) * 512])

                    # ---- V projection (fp8 DoubleRow), w-major rows ----
                    for wg in range(8 if "v" in parts else 0):
                        psv = psum_pool.tile([128, HD], F32, tag="pp")
                        for cp in range(8):
                            nc.tensor.matmul(
                                psv[:],
                                xr8t[:, cp * 2 * HW:(cp + 1) * 2 * HW].rearrange(
                                    "p (t n) -> p t n", t=2)[:, :, wg * 128:(wg + 1) * 128],
                                swv[:, cp * 512:(cp + 1) * 512].rearrange(
                                    "p (t n) -> p t n", t=2),
                                start=(cp == 0), stop=(cp == 7), perf_mode=DR)
                        nc.scalar.activation(sv[:, wg * HD:(wg + 1) * HD], psv[:],
                                             mybir.ActivationFunctionType.Copy,
                                             scale=float(1.0 / FS))

                    # ---- attention, software-pipelined over w-groups:
                    # PE order sc(0), sc(1), av(0), sc(2), av(1), ... so the
                    # softmax chain (ACT exp -> DVE recip/mul/transpose) of
                    # group wg hides behind sc matmuls of group wg+1 ----
                    if "att" in parts:
                        def sc_stage(wg):
                            sc = psumb_pool.tile([128, 128], F32, tag="sa")
                            nc.tensor.matmul(sc[:],
                                             qall[:, (2 * n) * HW + wg * 128:
                                                  (2 * n) * HW + (wg + 1) * 128],
                                             sk[:, wg * 128:(wg + 1) * 128],
                                             start=True, stop=False)
                            nc.tensor.matmul(sc[:],
                                             qall[:, (2 * n + 1) * HW + wg * 128:
                                                  (2 * n + 1) * HW + (wg + 1) * 128],
                                             sk[:, HW + wg * 128: HW + (wg + 1) * 128],
                                             start=False, stop=False)
                            nc.tensor.matmul(sc[:],
                                             sqa_all[:, n * HW + wg * 128:
                                                     n * HW + (wg + 1) * 128],
                                             c_kaug[:, wg * 128:(wg + 1) * 128],
                                             start=False, stop=True)
                            return sc

                        def av_stage(wg, sc):
                            probs = probs_pool.tile([128, 128], BF16, tag="pr")
                            sums = probs_pool.tile([128, 1], F32, tag="sm")
                            recip = probs_pool.tile([128, 1], F32, tag="rc")
                            nc.scalar.activation(probs[:], sc[:], EXP,
                                                 scale=float(1.0 / (16.0 * FS)),
                                                 accum_out=sums[:])
                            nc.vector.reciprocal(recip[:], sums[:])
                            nc.vector.tensor_scalar_mul(probs[:], probs[:], recip[:])
                            probsT = probs_pool.tile([128, 128], BF16, tag="prT")
                            nc.vector.transpose(probsT[:], probs[:])
                            av = psumb_pool.tile([128, 256], F32, tag="av")
                            for ch in range(2):
                                nc.tensor.matmul(
                                    av[:, ch * 128:(ch + 1) * 128],
                                    sv[:, wg * HD + ch * 128: wg * HD + (ch + 1) * 128],
                                    probsT[:], start=True, stop=True)
                            if 2 * n < OSPLIT:
                                dstT = attT8[:, 2 * n * HW:(2 * n + 2) * HW]
                            else:
                                base = (2 * n - OSPLIT) * HW
                                dstT = attTb[:, base:base + 2 * HW]
                            nc.vector.tensor_copy(
                                dstT.rearrange(
                                    "p (ch hw) -> p ch hw", ch=2)[:, :, wg * 128:(wg + 1) * 128],
                                av[:].rearrange("p (ch m) -> p ch m", ch=2))

                        prev = None
                        for wg in range(8):
                            sc = sc_stage(wg)
                            if prev is not None:
                                av_stage(wg - 1, prev)
                            prev = sc
                        av_stage(7, prev)

                # ---- output projection: contraction chunks 0..OSPLIT-1 in
                # fp8 DoubleRow (attT8 x 64*Wo8), the rest bf16 (x 64*Wo), all
                # accumulating into one 64x-scaled PSUM group; the drain
                # applies 1/64 and the bias in one tensor_scalar ----
                for co in range(NC if "o" in parts else 0):
                    wt8o = wstr_pool.tile([128, OSPLIT * 128], F8, tag="wo8")
                    nc.sync.dma_start(wt8o[:], wo8_l[co])
                    wtb = wstr_pool.tile([128, (NC - OSPLIT) * 128], BF16, tag="wob")
                    nc.sync.dma_start(wtb[:], wob_l[co])
                    for h2 in range(2):
                        ps = psum_pool.tile([128, 512], F32, tag="pp")
                        for cp in range(OSPLIT // 2):
                            nc.tensor.matmul(
                                ps[:],
                                wt8o[:, cp * 256:(cp + 1) * 256].rearrange(
                                    "p (t m) -> p t m", t=2),
                                attT8[:, cp * 2 * HW:(cp + 1) * 2 * HW].rearrange(
                                    "p (t nn) -> p t nn", t=2)[:, :, h2 * 512:(h2 + 1) * 512],
                                start=(cp == 0), stop=False, perf_mode=DR)
                        for j in range(NC - OSPLIT):
                            nc.tensor.matmul(
                                ps[:], wtb[:, j * 128:(j + 1) * 128],
                                attTb[:, j * HW + h2 * 512: j * HW + (h2 + 1) * 512],
                                start=False, stop=(j == NC - OSPLIT - 1))
                        ot = outs_pool.tile([128, 512], BF16, tag="ot")
                        nc.vector.tensor_scalar(
                            ot[:], ps[:], float(1.0 / FS), c_bo[:, co:co + 1],
                            mybir.AluOpType.mult, mybir.AluOpType.add)
                        nc.sync.dma_start(
                            out[co * 128:(co + 1) * 128, h2 * 512:(h2 + 1) * 512], ot[:])

                if timing_twin:
                    tt = outs_pool.tile([1, 4], F32, tag="tt")
                    nc.sync.dma_start(tt[:], tiny_in[:])
                    nc.sync.dma_start(tiny_out[:], tt[:])

            if timing_twin:
                tt = outs_pool.tile([1, 4], F32, tag="tt")
                nc.sync.dma_start(tt[:], tiny_in[:])
                nc.sync.dma_start(tiny_out[:], tt[:])

    nc.finalize()
    return nc


def kernel(left_features, right_features, Wq, bq, Wk, bk, Wv, bv, Wo, bo, rel_emb,
           _trace=False):
    from concourse.bass_utils import run_bass_kernel_spmd

    bf = ml_dtypes.bfloat16
    f8 = ml_dtypes.float8_e4m3
    if "nc" not in _CACHE:
        _CACHE["nc"] = _build()
    nc = _CACHE["nc"]

    consts = _hostprep(Wq, bq, Wk, bk, Wv, bv, Wo, bo, rel_emb)
    lf = np.asarray(left_features, np.float32)
    rf = np.asarray(right_features, np.float32)

    def wmajor(x, dt):  # (C, H, W) -> (C, HW) with col = w*32 + i
        return np.ascontiguousarray(x.transpose(0, 2, 1).reshape(C, HW)).astype(dt)

    in_maps = []
    for core in range(8):
        d, b = divmod(core, 4)
        qf = lf[b] if d == 0 else rf[b]
        rfb = rf[b] if d == 0 else lf[b]
        m = dict(consts)
        m["xq"] = wmajor(qf, bf)
        m["xr8"] = wmajor(rfb, f8)
        in_maps.append(m)

    res = run_bass_kernel_spmd(nc, in_maps, list(range(8)), trace=_trace)
    _CACHE["last_result"] = res

    def unperm(o):  # [C, HW w-major] -> (C, H, W)
        return np.ascontiguousarray(
            o.reshape(C, W, H).transpose(0, 2, 1)).astype(np.float32)

    wr = np.stack([unperm(res.results[b]["out"]) for b in range(4)])
    wl = np.stack([unperm(res.results[4 + b]["out"]) for b in range(4)])
    left_att = np.concatenate([lf, wr], axis=1)
    right_att = np.concatenate([rf, wl], axis=1)
    return (left_att, right_att)
